# revision 5
# baseline (speedup 1.0000x reference)
"""DiffPool batched-graph layer on 8 Trainium2 NeuronCores.

Strategy: shard the 64 graphs across 8 cores (8 graphs each). The
edge-list message passing is reformulated as dense linear algebra by
building the per-graph adjacency-count matrix A[dst,src] on the host
(a pure re-encoding of the integer edge list):

    agg   = (A @ h) / max(deg,1)          -> matmul with Anorm
    z     = [h, agg] @ Wcat (+ b)         -> matmul (+ rank-1 bias matmul)
    feat  = relu(z_f / max(||z_f||, eps))
    asg   = softmax(relu(z_p / max(||z_p||, eps)))
    hpool = asg^T @ feat
    blocks= asg^T A asg = asg^T @ (deg * (Anorm @ asg))

The dense block-diagonal adj_new (8192x8192, ~256 MB of mostly zeros)
is assembled host-side from the per-graph 128x128 blocks.

Matmul operands are fp16 (products are exact, accumulation is f32 in
PSUM); statistics (norms, softmax sums) and outputs are f32.
"""

import numpy as np
from contextlib import ExitStack

import concourse.bass as bass
import concourse.tile as tile
from concourse import bacc, mybir
from concourse.bass_utils import run_bass_kernel_spmd

F32 = mybir.dt.float32
F16 = mybir.dt.float16
AF = mybir.ActivationFunctionType
ALU = mybir.AluOpType

B, N, DIN, K, E = 64, 256, 256, 128, 8192
NCORES = 8
GPC = B // NCORES  # graphs per core
EPS = 1e-12

_CACHE = {}


def _build_nc(with_bias):
    nc = bacc.Bacc("TRN2", target_bir_lowering=False, debug=False)

    h_d = nc.dram_tensor("h8", [GPC, N, DIN], F16, kind="ExternalInput")    # [g, src, d]
    ht_d = nc.dram_tensor("ht8", [GPC, DIN, N], F16, kind="ExternalInput")  # [g, d, node]
    an_d = nc.dram_tensor("an8", [GPC, N, N], F16, kind="ExternalInput")    # [g, src, dst] = Anorm^T
    dg_d = nc.dram_tensor("dg8", [128, 2 * GPC], F32, kind="ExternalInput") # [p, g*2+c] = max(deg,1)
    wc_d = nc.dram_tensor("wc", [128, 4, 2 * K], F16, kind="ExternalInput") # Wcat chunks
    if with_bias:
        bc_d = nc.dram_tensor("bc", [1, 2 * K], F16, kind="ExternalInput")  # [b_feat|b_pool]
    blk_d = nc.dram_tensor("blk", [GPC, K, K], F32, kind="ExternalOutput")
    hp_d = nc.dram_tensor("hp", [GPC, K, K], F32, kind="ExternalOutput")

    with tile.TileContext(nc) as tc, ExitStack() as ctx:
        consts = ctx.enter_context(tc.tile_pool(name="consts", bufs=1))
        loads = ctx.enter_context(tc.tile_pool(name="loads", bufs=4))
        keep = ctx.enter_context(tc.tile_pool(name="keep", bufs=GPC))
        work = ctx.enter_context(tc.tile_pool(name="work", bufs=3))
        small = ctx.enter_context(tc.tile_pool(name="small", bufs=4))
        pp = ctx.enter_context(tc.tile_pool(name="pp", bufs=2, space="PSUM"))

        wc_sb = consts.tile([128, 4, 2 * K], F16)
        nc.sync.dma_start(wc_sb[:], wc_d[:])
        dg_sb = consts.tile([128, 2 * GPC], F32)
        nc.sync.dma_start(dg_sb[:], dg_d[:])
        if with_bias:
            bc_sb = consts.tile([1, 2 * K], F16)
            nc.sync.dma_start(bc_sb[:], bc_d[:])
            ones_sb = consts.tile([1, 128], F16)
            nc.vector.memset(ones_sb[:], 1.0)

        h_sb, ht_sb, an_sb, at_sb = [], [], [], []
        rhs2_sb, as_sb = [], []

        # ---- P1: load + aggT = h^T @ Anorm^T ----
        for g in range(GPC):
            h_t = loads.tile([128, 2, DIN], F16, tag="h", name=f"h_{g}")
            nc.sync.dma_start(h_t[:], h_d[g].rearrange("(c p) d -> p c d", p=128))
            ht_t = keep.tile([128, 2, N], F16, tag="ht", name=f"ht_{g}")
            nc.sync.dma_start(ht_t[:], ht_d[g].rearrange("(c p) n -> p c n", p=128))
            an_t = keep.tile([128, 2, N], F16, tag="an", name=f"an_{g}")
            nc.sync.dma_start(an_t[:], an_d[g].rearrange("(c p) n -> p c n", p=128))
            h_sb.append(h_t); ht_sb.append(ht_t); an_sb.append(an_t)

            at_t = keep.tile([128, 2, N], F16, tag="at", name=f"at_{g}")
            for t in range(2):  # d-tile
                agg_ps = pp.tile([128, N], F32, tag="aggout", name=f"agg_{g}_{t}")
                for c in range(2):  # src chunk
                    nc.tensor.matmul(
                        agg_ps[:],
                        h_t[:, c, t * 128:(t + 1) * 128],
                        an_t[:, c, :],
                        start=(c == 0), stop=(c == 1),
                    )
                nc.vector.tensor_copy(at_t[:, t, :], agg_ps[:])
            at_sb.append(at_t)

        # ---- P2: z = [h|agg] @ Wcat (+ b); normalize; softmax ----
        for g in range(GPC):
            rhs2_t = keep.tile([128, 2, 2 * K], F16, tag="rhs2", name=f"rhs2_{g}")
            as_t = keep.tile([128, 2, K], F16, tag="asg", name=f"as_{g}")
            rhs2_sb.append(rhs2_t); as_sb.append(as_t)
            for t in range(2):  # node tile
                z_ps = pp.tile([128, 2 * K], F32, tag="z", name=f"z_{g}_{t}")
                for c in range(4):
                    lhs = ht_sb[g] if c < 2 else at_sb[g]
                    nc.tensor.matmul(
                        z_ps[:],
                        lhs[:, c % 2, t * 128:(t + 1) * 128],
                        wc_sb[:, c, :],
                        start=(c == 0), stop=(not with_bias and c == 3),
                    )
                if with_bias:
                    nc.tensor.matmul(z_ps[:], ones_sb[:], bc_sb[:],
                                     start=False, stop=True)

                sq = work.tile([128, 2 * K], F32, tag="sq", name=f"sq_{g}_{t}")
                # --- feat half: feat = relu(z / max(||z||, eps)) ---
                ssf = small.tile([128, 1], F32, tag="ssf", name=f"ssf_{g}_{t}")
                nc.scalar.activation(sq[:, 0:K], z_ps[:, 0:K], AF.Square,
                                     accum_out=ssf[:])
                nrf = small.tile([128, 1], F32, tag="nrf", name=f"nrf_{g}_{t}")
                nc.scalar.sqrt(nrf[:], ssf[:])
                nc.vector.tensor_scalar_max(nrf[:], nrf[:], EPS)
                rif = small.tile([128, 1], F32, tag="rif", name=f"rif_{g}_{t}")
                nc.vector.reciprocal(rif[:], nrf[:])
                nc.scalar.activation(rhs2_t[:, t, K:2 * K], z_ps[:, 0:K],
                                     AF.Relu, scale=rif[:])
                # --- pool half: asg = softmax(relu(z / max(||z||, eps))) ---
                ssp = small.tile([128, 1], F32, tag="ssp", name=f"ssp_{g}_{t}")
                nc.scalar.activation(sq[:, K:2 * K], z_ps[:, K:2 * K], AF.Square,
                                     accum_out=ssp[:])
                nrp = small.tile([128, 1], F32, tag="nrp", name=f"nrp_{g}_{t}")
                nc.scalar.sqrt(nrp[:], ssp[:])
                nc.vector.tensor_scalar_max(nrp[:], nrp[:], EPS)
                rip = small.tile([128, 1], F32, tag="rip", name=f"rip_{g}_{t}")
                nc.vector.reciprocal(rip[:], nrp[:])
                # exp(relu(x)) = max(exp(x), 1) for x in [-1, 1]
                er = work.tile([128, K], F32, tag="er", name=f"er_{g}_{t}")
                nc.scalar.activation(er[:], z_ps[:, K:2 * K], AF.Exp, scale=rip[:])
                em = work.tile([128, K], F32, tag="em", name=f"em_{g}_{t}")
                es = small.tile([128, 1], F32, tag="es", name=f"es_{g}_{t}")
                nc.vector.tensor_scalar_max(em[:], er[:], 1.0)
                nc.vector.reduce_sum(es[:], em[:], axis=mybir.AxisListType.X)
                rs = small.tile([128, 1], F32, tag="rs", name=f"rs_{g}_{t}")
                nc.vector.reciprocal(rs[:], es[:])
                nc.vector.tensor_scalar_mul(as_t[:, t, :], em[:], rs[:])

        # ---- P3: w = Anorm @ assign, scaled by deg ----
        for g in range(GPC):
            for t in range(2):  # dst tile
                w_ps = pp.tile([128, K], F32, tag="w", name=f"w_{g}_{t}")
                for c in range(2):  # src chunk
                    nc.tensor.matmul(
                        w_ps[:],
                        an_sb[g][:, c, t * 128:(t + 1) * 128],
                        as_sb[g][:, c, :],
                        start=(c == 0), stop=(c == 1),
                    )
                nc.vector.tensor_scalar_mul(
                    rhs2_sb[g][:, t, 0:K], w_ps[:],
                    dg_sb[:, 2 * g + t: 2 * g + t + 1])

        # ---- P4: [blocks | hpool] = assign^T @ [w_sc | feat] ----
        for g in range(GPC):
            o_ps = pp.tile([128, 2 * K], F32, tag="aggout", name=f"o_{g}")
            for c in range(2):  # node chunk
                nc.tensor.matmul(
                    o_ps[:],
                    as_sb[g][:, c, :],
                    rhs2_sb[g][:, c, :],
                    start=(c == 0), stop=(c == 1),
                )
            o_sb = work.tile([128, 2 * K], F32, tag="osb", name=f"o_sb_{g}")
            nc.vector.tensor_copy(o_sb[:], o_ps[:])
            nc.sync.dma_start(blk_d[g], o_sb[:, 0:K])
            nc.sync.dma_start(hp_d[g], o_sb[:, K:2 * K])

    nc.compile()
    return nc


def _get_nc(with_bias):
    key = ("nc", with_bias)
    if key not in _CACHE:
        _CACHE[key] = _build_nc(with_bias)
    return _CACHE[key]


def _prep(inputs):
    h = np.ascontiguousarray(np.asarray(inputs["h"], dtype=np.float32))
    es = np.asarray(inputs["edge_src"]).astype(np.int64)
    ed = np.asarray(inputs["edge_dst"]).astype(np.int64)
    Wf = np.asarray(inputs["W_feat"], dtype=np.float32)
    bf = np.asarray(inputs["b_feat"], dtype=np.float32)
    Wp = np.asarray(inputs["W_pool"], dtype=np.float32)
    bp = np.asarray(inputs["b_pool"], dtype=np.float32)

    # adjacency counts A[g, dst, src] from the edge list
    lin = (np.arange(B, dtype=np.int64)[:, None] * (N * N) + ed * N + es).ravel()
    A = np.bincount(lin, minlength=B * N * N).astype(np.float32).reshape(B, N, N)
    degM = np.maximum(A.sum(axis=2), 1.0)                      # [g, dst]
    AnT = (A / degM[:, :, None]).transpose(0, 2, 1).astype(np.float16)
    h16 = h.astype(np.float16)
    hT = np.ascontiguousarray(h16.transpose(0, 2, 1))
    h16 = np.ascontiguousarray(h16)
    AnT = np.ascontiguousarray(AnT)
    Wcat = np.concatenate([Wf, Wp], axis=1)                    # [512, 256]
    wc = np.ascontiguousarray(
        Wcat.reshape(4, 128, 2 * K).transpose(1, 0, 2).astype(np.float16))
    bc = np.ascontiguousarray(np.concatenate([bf, bp])[None, :].astype(np.float16))
    with_bias = bool(np.any(bc))

    in_maps = []
    for c in range(NCORES):
        sl = slice(c * GPC, (c + 1) * GPC)
        dg = np.ascontiguousarray(
            degM[sl].reshape(GPC, 2, 128).transpose(2, 0, 1).reshape(128, 2 * GPC))
        m = {"h8": h16[sl], "ht8": hT[sl], "an8": AnT[sl], "dg8": dg, "wc": wc}
        if with_bias:
            m["bc"] = bc
        in_maps.append(m)
    return in_maps, with_bias


def run(inputs, trace=False, tmpdir=None):
    in_maps, with_bias = _prep(inputs)
    nc = _get_nc(with_bias)
    res = run_bass_kernel_spmd(
        nc, in_maps, core_ids=list(range(NCORES)), trace=trace, tmpdir=tmpdir)

    blocks = np.concatenate([res.results[c]["blk"] for c in range(NCORES)], axis=0)
    hpool = np.concatenate([res.results[c]["hp"] for c in range(NCORES)], axis=0)

    adj = np.zeros((B * K, B * K), dtype=np.float32)
    for g in range(B):
        adj[g * K:(g + 1) * K, g * K:(g + 1) * K] = blocks[g]
    return (adj, hpool.reshape(B * K, K)), res


def kernel(**inputs):
    out, _ = run(inputs, trace=False)
    return out
